# revision 22
# baseline (speedup 1.0000x reference)
"""Trainium2 Bass kernel for nn_AnswerModule (scatter_memory, 8 cores).

Strategy: pure data-parallel over batch (4 examples per core).  The
heavy einsums are collapsed via matmul associativity:
  p1 = softmax((s@W6) @ M),  attn@W7b = p1 @ (M^T @ W7b),
  p2 = softmax((s@W7t + attn@W7b) @ M).
The tiny GRU / alpha-attention recurrence runs on host.

Device dataflow (per core) is built around LONG moving-dim matmuls
(512 cols) with tiny stationaries to minimize PE instruction count
(per-instruction overhead on TRN2 is ~130-350 ns).  All partition
bases are 32-aligned (hardware requirement for every engine):
  (a) l1 logits: stationary = SW6 (128d x 4), moving = M (128d x 512),
      into (4,512) psum wave tiles; evacuation IS the exp: scalar
      activations write exp1[32b+t, n] with accum_out Z-partials.
  (b) attn u: host uploads M^T (n-major); stationary = exp1^T chunk
      slices (128n x 4) from 32 PE transposes, moving = M^T
      (128n x 256), accumulated over n into psum (4,256) per example.
  (c) l2 logits: same wave scheme as (a) with v2^T stationaries.
  Finals: 8 accumulating matmuls per output with per-block masked
  block-diag stationaries bd3_g[32b+t, 4b'+g'] = rz[b,t] d(b=b') d(g=g'),
  then one LOG activation and a partition-contiguous DMA.
"""

import sys

sys.path.insert(0, "/opt/trn_rl_repo")

import numpy as np
import ml_dtypes

import concourse.bass as bass
import concourse.bacc as bacc
import concourse.mybir as mybir
from concourse import tile
from concourse.bass_utils import run_bass_kernel_spmd

B, QL, PL, T, D2 = 32, 64, 4096, 4, 256
NCORES = 8
BL = B // NCORES  # 4 examples per core
NG = 8  # n-groups of 512
F32 = mybir.dt.float32
F32R = mybir.dt.float32r
BF16 = mybir.dt.bfloat16

_NC = None


def _build_graph():
    nc = bacc.Bacc("TRN2", target_bir_lowering=False, debug=False)

    # DRAM inputs
    m_d = nc.dram_tensor("m", [BL, 2, 128, PL], F32R, kind="ExternalInput").ap()
    mt_d = nc.dram_tensor("mt", [BL, 4, 128, 2048], BF16, kind="ExternalInput").ap()
    sw6_d = nc.dram_tensor("sw6", [2, 128, 16], F32R, kind="ExternalInput").ap()
    v1t_d = nc.dram_tensor("v1t", [2, 128, 16], F32R, kind="ExternalInput").ap()
    w7b_d = nc.dram_tensor("w7b", [2, 128, 256], F32R, kind="ExternalInput").ap()
    eye_d = nc.dram_tensor("eye", [128, 128], F32R, kind="ExternalInput").ap()
    p4g_d = nc.dram_tensor("p4g", [128, 256], F32R, kind="ExternalInput").ap()
    o1_d = nc.dram_tensor("o1", [BL, PL], F32, kind="ExternalOutput").ap()
    o2_d = nc.dram_tensor("o2", [BL, PL], F32, kind="ExternalOutput").ap()

    EXP = mybir.ActivationFunctionType.Exp
    LOG = getattr(mybir.ActivationFunctionType, "Log", None) or getattr(
        mybir.ActivationFunctionType, "Ln"
    )

    with tile.TileContext(nc) as tc:
        with (
            nc.allow_low_precision(reason="float32r is 4-byte, same width as f32"),
            tc.tile_pool(name="const", bufs=1) as cpool,
            tc.tile_pool(name="m", bufs=1) as mpool,
            tc.tile_pool(name="mt", bufs=4) as mtpool,
            tc.tile_pool(name="sb", bufs=1) as spool,
            tc.tile_pool(name="mega", bufs=3, space="PSUM") as megapool,
            tc.tile_pool(name="pst", bufs=2, space="PSUM") as pstpool,
            tc.tile_pool(name="psu", bufs=1, space="PSUM") as psupool,
            tc.tile_pool(name="sm", bufs=2, space="PSUM") as smpool,
        ):
            # ---- small constants ----
            sw6_sb = cpool.tile([128, 32], F32R, tag="sw6")
            v1t_sb = cpool.tile([128, 32], F32R, tag="v1t")
            for dc in range(2):
                nc.sync.dma_start(
                    out=sw6_sb[:, 16 * dc : 16 * dc + 16], in_=sw6_d[dc]
                )
                nc.sync.dma_start(
                    out=v1t_sb[:, 16 * dc : 16 * dc + 16], in_=v1t_d[dc]
                )
            w7b_sb = []
            for h in range(2):
                t_ = cpool.tile([128, 256], F32R, tag=f"w7b{h}", name=f"w7b{h}")
                nc.sync.dma_start(out=t_[:], in_=w7b_d[h])
                w7b_sb.append(t_)
            eye_sb = cpool.tile([128, 128], F32R, tag="eye")
            nc.sync.dma_start(out=eye_sb[:], in_=eye_d[:, :])
            p4g_sb = cpool.tile([128, 256], F32R, tag="p4g")
            nc.sync.dma_start(out=p4g_sb[:], in_=p4g_d[:, :])

            # exp stacks: rows 32b + t, cols n.  Junk rows zeroed once.
            exp1_sb = spool.tile([128, PL], F32R, tag="exp1")
            exp2_sb = spool.tile([128, PL], F32R, tag="exp2")
            nc.gpsimd.memset(exp1_sb[:].bitcast(F32), 1.0)
            nc.gpsimd.memset(exp2_sb[:].bitcast(F32), 1.0)

            # ---- M resident tiles (2MB each), then MT streaming tiles ----
            m_sb = [[None, None] for _ in range(BL)]
            for b in range(BL):
                for dc in range(2):
                    mtile = mpool.tile(
                        [128, PL], F32R, tag=f"m{b}_{dc}", name=f"m{b}_{dc}"
                    )
                    nc.sync.dma_start(out=mtile[:], in_=m_d[b, dc])
                    m_sb[b][dc] = mtile
            # MT: per example 4 tiles of (128, 8*256); issued after M so M
            # wins the DMA bandwidth early.
            mt_tiles = [[None] * 4 for _ in range(BL)]
            for b in range(BL):
                for kg in range(4):
                    t_ = mtpool.tile([128, 2048], BF16, tag="mt", name=f"mt{b}_{kg}")
                    nc.sync.dma_start(out=t_[:], in_=mt_d[b, kg])
                    mt_tiles[b][kg] = t_

            # ---- (a) l1 logits via psum wave tiles; evac = EXP act ----
            for b in range(BL):
                for g in range(NG):
                    wave = megapool.tile(
                        [4, 512], F32, tag="mega", name=f"wa{b}_{g}"
                    )
                    for dc in range(2):
                        nc.tensor.matmul(
                            wave[:],
                            sw6_sb[:, 16 * dc + 4 * b : 16 * dc + 4 * b + 4],
                            m_sb[b][dc][:, 512 * g : 512 * g + 512],
                            start=(dc == 0),
                            stop=(dc == 1),
                        )
                    nc.scalar.activation(
                        exp1_sb[32 * b : 32 * b + 4, 512 * g : 512 * g + 512],
                        wave[:],
                        EXP,
                    )

            # Z and 1/Z at rows 32b + t
            z1 = spool.tile([128, 1], F32, tag="z1")
            nc.vector.tensor_reduce(
                z1[:], exp1_sb[:], axis=mybir.AxisListType.X, op=mybir.AluOpType.add
            )
            rz1 = spool.tile([128, 1], F32, tag="rz1")
            nc.vector.reciprocal(rz1[:], z1[:])
            bd31 = spool.tile([128, 256], F32R, tag="bd31")
            for g in range(NG):
                nc.vector.tensor_scalar_mul(
                    bd31[:, 32 * g : 32 * g + 32],
                    p4g_sb[:, 32 * g : 32 * g + 32],
                    rz1[:],
                )

            # ---- exp1^T via 32 transposes: T_k at exp1T[:, 128k:+128] ----
            # stationary for (b, chunk k) = exp1T[:, 128k + 32b : +4]
            exp1T = spool.tile([128, PL], BF16, tag="exp1T")
            for k in range(32):
                pst = pstpool.tile([128, 128], F32R, tag="pst", name=f"pst{k}")
                nc.tensor.transpose(
                    pst[:], exp1_sb[:, 128 * k : 128 * k + 128], eye_sb[:]
                )
                nc.vector.tensor_copy(exp1T[:, 128 * k : 128 * k + 128], pst[:])

            # ---- per-example: u (attn), cav, v2, l2 ----
            uT = spool.tile([128, 32], F32R, tag="uT")
            v2t_sb = spool.tile([128, 32], F32R, tag="v2t")
            for b in range(BL):
                # (b) u_b[t, e] = sum_n exp1T_b[n, t] * MT_b[n, e]  -> (4, 256)
                psu = psupool.tile([4, 256], F32, tag="psu", name=f"psu{b}")
                for k in range(32):
                    kg, kk = k // 8, k % 8
                    nc.tensor.matmul(
                        psu[:],
                        exp1T[:, 128 * k + 32 * b : 128 * k + 32 * b + 4],
                        mt_tiles[b][kg][:, 256 * kk : 256 * kk + 256],
                        start=(k == 0),
                        stop=(k == 31),
                    )
                # extract + scale by 1/Z1 (double-buffered (4,256) tile)
                u_pack = spool.tile(
                    [4, 256], F32R, tag="u_pack", bufs=2, name=f"up{b}"
                )
                nc.vector.tensor_scalar_mul(
                    u_pack[:], psu[:], rz1[32 * b : 32 * b + 4, :]
                )
                # uT via 2 pack-transposes (4,128) -> (128,4)
                for h in range(2):
                    pstu = pstpool.tile([128, 4], F32R, tag="pst", name=f"pu{b}{h}")
                    nc.tensor.transpose(
                        pstu[:],
                        u_pack[:, 128 * h : 128 * h + 128],
                        eye_sb[0:4, 0:4],
                    )
                    nc.vector.tensor_copy(
                        uT[:, 8 * b + 4 * h : 8 * b + 4 * h + 4], pstu[:]
                    )
                # cavT[m] = sum_h W7b[h][:, m-chunk]^T @ uT[h]  -> (128, 4)
                for m in range(2):
                    cps = smpool.tile([128, 4], F32, tag="sm", name=f"cv{b}{m}")
                    for h in range(2):
                        nc.tensor.matmul(
                            cps[:],
                            w7b_sb[h][:, 128 * m : 128 * m + 128],
                            uT[:, 8 * b + 4 * h : 8 * b + 4 * h + 4],
                            start=(h == 0),
                            stop=(h == 1),
                        )
                    nc.vector.tensor_add(
                        v2t_sb[:, 16 * m + 4 * b : 16 * m + 4 * b + 4],
                        cps[:],
                        v1t_sb[:, 16 * m + 4 * b : 16 * m + 4 * b + 4],
                    )
                # (c) l2 logits for example b; evac = EXP act
                for g in range(NG):
                    wave = megapool.tile(
                        [4, 512], F32, tag="mega", name=f"wc{b}_{g}"
                    )
                    for dc in range(2):
                        nc.tensor.matmul(
                            wave[:],
                            v2t_sb[:, 16 * dc + 4 * b : 16 * dc + 4 * b + 4],
                            m_sb[b][dc][:, 512 * g : 512 * g + 512],
                            start=(dc == 0),
                            stop=(dc == 1),
                        )
                    nc.scalar.activation(
                        exp2_sb[32 * b : 32 * b + 4, 512 * g : 512 * g + 512],
                        wave[:],
                        EXP,
                    )

            # ---- softmax2 normalizers ----
            z2 = spool.tile([128, 1], F32, tag="z2")
            nc.vector.tensor_reduce(
                z2[:], exp2_sb[:], axis=mybir.AxisListType.X, op=mybir.AluOpType.add
            )
            rz2 = spool.tile([128, 1], F32, tag="rz2")
            nc.vector.reciprocal(rz2[:], z2[:])
            bd32 = spool.tile([128, 256], F32R, tag="bd32")
            for g in range(NG):
                nc.vector.tensor_scalar_mul(
                    bd32[:, 32 * g : 32 * g + 32],
                    p4g_sb[:, 32 * g : 32 * g + 32],
                    rz2[:],
                )

            # ---- final p1/p2: 8 accumulating matmuls each + log + DMA ----
            # fin rows 8b + g ; cols c -> n = 512g + c
            for which, exp_sb, bd, o_d in (
                (0, exp1_sb, bd31, o1_d),
                (1, exp2_sb, bd32, o2_d),
            ):
                fin = megapool.tile([32, 512], F32, tag="mega", name=f"fin{which}")
                for g in range(NG):
                    nc.tensor.matmul(
                        fin[:],
                        bd[:, 32 * g : 32 * g + 32],
                        exp_sb[:, 512 * g : 512 * g + 512],
                        start=(g == 0),
                        stop=(g == NG - 1),
                    )
                o_sb = spool.tile([32, 512], F32, tag=f"o{which}", name=f"o{which}")
                nc.scalar.activation(o_sb[:], fin[:], LOG, scale=1.0 / PL)
                nc.sync.dma_start(
                    out=o_d.rearrange("b (g c) -> (b g) c", g=NG), in_=o_sb[:]
                )

    nc.compile()
    return nc


def _host_precompute(inp):
    H_q, M, W_4, W_6, W_7 = (
        inp["H_q"],
        inp["M"],
        inp["W_4"],
        inp["W_6"],
        inp["W_7"],
    )
    wih, whh, bih, bhh = (
        inp["gru_w_ih"],
        inp["gru_w_hh"],
        inp["gru_b_ih"],
        inp["gru_b_hh"],
    )
    lg = H_q @ W_4
    a = np.exp(lg - lg.max(1, keepdims=True))
    a /= a.sum(1, keepdims=True)
    s = np.einsum("bq,bqh->bh", a, H_q).astype(np.float32)
    x = M.mean(axis=2)
    gh = x @ whh.T + bhh
    ghr, ghz, ghn = np.split(gh, 3, axis=1)
    s_all = [s]
    for _ in range(T - 1):
        gi = s @ wih.T + bih
        gir, giz, gin = np.split(gi, 3, axis=1)
        r = 1.0 / (1.0 + np.exp(-(gir + ghr)))
        z = 1.0 / (1.0 + np.exp(-(giz + ghz)))
        n = np.tanh(gin + r * ghn)
        s = (1.0 - z) * n + z * x
        s_all.append(s)
    S = np.stack(s_all).astype(np.float32)  # (T, B, D2)
    SW6 = np.einsum("tbd,de->tbe", S, W_6).astype(np.float32)
    W7t, W7b = W_7[:D2], W_7[D2:]
    V1 = np.einsum("tbd,de->tbe", S, W7t).astype(np.float32)
    return SW6, V1, W7b


def kernel(**inputs):
    global _NC
    inp = {
        k: np.ascontiguousarray(np.asarray(v, dtype=np.float32))
        for k, v in inputs.items()
    }
    SW6, V1, W7b = _host_precompute(inp)
    M = inp["M"]

    eye = np.eye(128, dtype=np.float32)
    # p4g[:, 32g:32g+32][32b+t, 8b'+g'] = d(b=b') * d(g=g')  (t < 4 rows only)
    rows = np.arange(128)
    bb, tt = rows // 32, rows % 32
    p4g = np.zeros((128, 256), dtype=np.float32)
    for g in range(NG):
        valid = tt < 4
        p4g[rows[valid], 32 * g + 8 * bb[valid] + g] = 1.0
    w7b_h = np.ascontiguousarray(W7b.reshape(2, 128, 256))

    if _NC is None:
        _NC = _build_graph()
    in_maps = []
    for i in range(NCORES):
        sl = slice(i * BL, (i + 1) * BL)
        Mc = M[sl]  # (BL, 256, PL)
        # mt[b, kg, p, 256*kk + e] = M[b, e, 1024*kg + 128*kk + p]
        mt = np.ascontiguousarray(
            Mc.transpose(0, 2, 1)
            .reshape(BL, 4, 8, 128, 256)
            .transpose(0, 1, 3, 2, 4)
            .reshape(BL, 4, 128, 2048)
            .astype(ml_dtypes.bfloat16)
        )
        mc = np.ascontiguousarray(Mc.reshape(BL, 2, 128, PL))
        # sw6/v1t: [dc, d', 4b+t] = X[t, b, 128dc + d']
        sw6c = np.ascontiguousarray(
            SW6[:, sl].transpose(2, 1, 0).reshape(2, 128, 16)
        )
        v1c = np.ascontiguousarray(
            V1[:, sl].transpose(2, 1, 0).reshape(2, 128, 16)
        )
        in_maps.append(
            {
                "m": mc,
                "mt": mt,
                "sw6": sw6c,
                "v1t": v1c,
                "w7b": w7b_h,
                "eye": eye,
                "p4g": p4g,
            }
        )
    global _LAST_IN_MAPS
    _LAST_IN_MAPS = in_maps
    res = run_bass_kernel_spmd(_NC, in_maps, core_ids=list(range(NCORES)))
    out1 = np.empty((B, PL), np.float32)
    out2 = np.empty((B, PL), np.float32)
    for i in range(NCORES):
        out1[i * BL : (i + 1) * BL] = res.results[i]["o1"]
        out2[i * BL : (i + 1) * BL] = res.results[i]["o2"]
    return out1, out2


# revision 24
# speedup vs baseline: 1.2348x; 1.2348x over previous
"""Trainium2 Bass kernel for nn_AnswerModule (scatter_memory, 8 cores).

Strategy: pure data-parallel over batch (4 examples per core).  The
heavy einsums are collapsed via matmul associativity:
  p1 = softmax((s@W6) @ M),  attn@W7b = p1 @ (M^T @ W7b),
  p2 = softmax((s@W7t + attn@W7b) @ M).
The tiny GRU / alpha-attention recurrence runs on host.

Device dataflow (per core) is built around LONG moving-dim matmuls
(512 cols) with tiny stationaries to minimize PE instruction count
(per-instruction overhead on TRN2 is ~130-350 ns).  All partition
bases are 32-aligned (hardware requirement for every engine):
  (a) l1 logits: stationary = SW6 (128d x 4), moving = M (128d x 512),
      into (4,512) psum wave tiles; evacuation IS the exp: scalar
      activations write exp1[32b+t, n] with accum_out Z-partials.
  (b) attn u: host uploads M^T (n-major); stationary = exp1^T chunk
      slices (128n x 4) from 32 PE transposes, moving = M^T
      (128n x 256), accumulated over n into psum (4,256) per example.
  (c) l2 logits: same wave scheme as (a) with v2^T stationaries.
  Finals: 8 accumulating matmuls per output with per-block masked
  block-diag stationaries bd3_g[32b+t, 4b'+g'] = rz[b,t] d(b=b') d(g=g'),
  then one LOG activation and a partition-contiguous DMA.
"""

import sys

sys.path.insert(0, "/opt/trn_rl_repo")

import numpy as np
import ml_dtypes

import concourse.bass as bass
import concourse.bacc as bacc
import concourse.mybir as mybir
from concourse import tile
from concourse.bass_utils import run_bass_kernel_spmd

B, QL, PL, T, D2 = 32, 64, 4096, 4, 256
NCORES = 8
BL = B // NCORES  # 4 examples per core
NG = 8  # n-groups of 512
F32 = mybir.dt.float32
F32R = mybir.dt.float32r
BF16 = mybir.dt.bfloat16

_NC = None


def _build_graph():
    nc = bacc.Bacc("TRN2", target_bir_lowering=False, debug=False)

    # DRAM inputs
    m_d = nc.dram_tensor("m", [BL, 2, 128, PL], F32R, kind="ExternalInput").ap()
    mt_d = nc.dram_tensor("mt", [BL, 4, 128, 2048], BF16, kind="ExternalInput").ap()
    sw6_d = nc.dram_tensor("sw6", [2, 128, 16], F32R, kind="ExternalInput").ap()
    v1t_d = nc.dram_tensor("v1t", [2, 128, 16], F32R, kind="ExternalInput").ap()
    w7b_d = nc.dram_tensor("w7b", [2, 128, 256], F32R, kind="ExternalInput").ap()
    eye_d = nc.dram_tensor("eye", [128, 128], F32R, kind="ExternalInput").ap()
    p4g_d = nc.dram_tensor("p4g", [128, 256], F32R, kind="ExternalInput").ap()
    o1_d = nc.dram_tensor("o1", [BL, PL], F32, kind="ExternalOutput").ap()
    o2_d = nc.dram_tensor("o2", [BL, PL], F32, kind="ExternalOutput").ap()

    EXP = mybir.ActivationFunctionType.Exp
    LOG = getattr(mybir.ActivationFunctionType, "Log", None) or getattr(
        mybir.ActivationFunctionType, "Ln"
    )

    with tile.TileContext(nc) as tc:
        with (
            nc.allow_low_precision(reason="float32r is 4-byte, same width as f32"),
            tc.tile_pool(name="const", bufs=1) as cpool,
            tc.tile_pool(name="m", bufs=1) as mpool,
            tc.tile_pool(name="mt", bufs=4) as mtpool,
            tc.tile_pool(name="sb", bufs=1) as spool,
            tc.tile_pool(name="mega", bufs=3, space="PSUM") as megapool,
            tc.tile_pool(name="pst", bufs=2, space="PSUM") as pstpool,
            tc.tile_pool(name="psu", bufs=2, space="PSUM") as psupool,
            tc.tile_pool(name="sm", bufs=1, space="PSUM") as smpool,
        ):
            # ---- small constants ----
            sw6_sb = cpool.tile([128, 32], F32R, tag="sw6")
            v1t_sb = cpool.tile([128, 32], F32R, tag="v1t")
            for dc in range(2):
                nc.sync.dma_start(
                    out=sw6_sb[:, 16 * dc : 16 * dc + 16], in_=sw6_d[dc]
                )
                nc.sync.dma_start(
                    out=v1t_sb[:, 16 * dc : 16 * dc + 16], in_=v1t_d[dc]
                )
            w7b_sb = []
            for h in range(2):
                t_ = cpool.tile([128, 256], F32R, tag=f"w7b{h}", name=f"w7b{h}")
                nc.sync.dma_start(out=t_[:], in_=w7b_d[h])
                w7b_sb.append(t_)
            eye_sb = cpool.tile([128, 128], F32R, tag="eye")
            nc.sync.dma_start(out=eye_sb[:], in_=eye_d[:, :])
            p4g_sb = cpool.tile([128, 256], F32R, tag="p4g")
            nc.sync.dma_start(out=p4g_sb[:], in_=p4g_d[:, :])

            # exp stacks: rows 32b + t, cols n.  Junk rows zeroed once.
            exp1_sb = spool.tile([128, PL], F32R, tag="exp1")
            exp2_sb = spool.tile([128, PL], F32R, tag="exp2")
            nc.gpsimd.memset(exp1_sb[:].bitcast(F32), 1.0)
            nc.gpsimd.memset(exp2_sb[:].bitcast(F32), 1.0)

            # ---- M resident tiles (2MB each), then MT streaming tiles ----
            m_sb = [[None, None] for _ in range(BL)]
            for b in range(BL):
                for dc in range(2):
                    mtile = mpool.tile(
                        [128, PL], F32R, tag=f"m{b}_{dc}", name=f"m{b}_{dc}"
                    )
                    nc.sync.dma_start(out=mtile[:], in_=m_d[b, dc])
                    m_sb[b][dc] = mtile
            # MT: per example 4 tiles of (128, 8*256); issued after M so M
            # wins the DMA bandwidth early.
            mt_tiles = [[None] * 4 for _ in range(BL)]
            for b in range(BL):
                for kg in range(4):
                    t_ = mtpool.tile([128, 2048], BF16, tag="mt", name=f"mt{b}_{kg}")
                    nc.sync.dma_start(out=t_[:], in_=mt_d[b, kg])
                    mt_tiles[b][kg] = t_

            # ---- (a) l1 logits via psum wave tiles; evac = EXP act ----
            exp1T = spool.tile([128, PL], BF16, tag="exp1T")

            def a_wave(b, g):
                wave = megapool.tile([4, 512], F32, tag="mega", name=f"wa{b}_{g}")
                for dc in range(2):
                    nc.tensor.matmul(
                        wave[:],
                        sw6_sb[:, 16 * dc + 4 * b : 16 * dc + 4 * b + 4],
                        m_sb[b][dc][:, 512 * g : 512 * g + 512],
                        start=(dc == 0),
                        stop=(dc == 1),
                    )
                nc.scalar.activation(
                    exp1_sb[32 * b : 32 * b + 4, 512 * g : 512 * g + 512],
                    wave[:],
                    EXP,
                )

            def transpose_chunk(k):
                # T_k at exp1T[:, 128k:+128]; stationary for (b, chunk k)
                # = exp1T[:, 128k + 32b : +4]
                pst = pstpool.tile([128, 128], F32R, tag="pst", name=f"pst{k}")
                nc.tensor.transpose(
                    pst[:], exp1_sb[:, 128 * k : 128 * k + 128], eye_sb[:]
                )
                nc.vector.tensor_copy(exp1T[:, 128 * k : 128 * k + 128], pst[:])

            for b in range(BL - 1):
                for g in range(NG):
                    a_wave(b, g)
            # last example: interleave the exp1^T transposes per finished group
            for g in range(NG):
                a_wave(BL - 1, g)
                for j in range(4):
                    transpose_chunk(4 * g + j)

            # Z and 1/Z at rows 32b + t
            z1 = spool.tile([128, 1], F32, tag="z1")
            nc.vector.tensor_reduce(
                z1[:], exp1_sb[:], axis=mybir.AxisListType.X, op=mybir.AluOpType.add
            )
            rz1 = spool.tile([128, 1], F32, tag="rz1")
            nc.vector.reciprocal(rz1[:], z1[:])
            bd31 = spool.tile([128, 256], F32R, tag="bd31")
            for g in range(NG):
                nc.vector.tensor_scalar_mul(
                    bd31[:, 32 * g : 32 * g + 32],
                    p4g_sb[:, 32 * g : 32 * g + 32],
                    rz1[:],
                )

            # ---- per-example u (attn) / cav / l2, software-pipelined so the
            # PE never stalls on the small vector chain of the previous
            # example ----
            uT = spool.tile([128, 32], F32R, tag="uT")
            v2t_sb = spool.tile([128, 32], F32R, tag="v2t")
            psus = [None] * BL

            def u_pass(b):
                # u_b[t, e] = sum_n exp1T_b[n, t] * MT_b[n, e]  -> (4, 256)
                psu = psupool.tile([4, 256], F32, tag="psu", name=f"psu{b}")
                for k in range(32):
                    kg, kk = k // 8, k % 8
                    nc.tensor.matmul(
                        psu[:],
                        exp1T[:, 128 * k + 32 * b : 128 * k + 32 * b + 4],
                        mt_tiles[b][kg][:, 256 * kk : 256 * kk + 256],
                        start=(k == 0),
                        stop=(k == 31),
                    )
                psus[b] = psu

            def cav_and_c(b):
                # extract + scale by 1/Z1 (double-buffered (4,256) tile)
                u_pack = spool.tile(
                    [4, 256], F32R, tag="u_pack", bufs=2, name=f"up{b}"
                )
                nc.vector.tensor_scalar_mul(
                    u_pack[:], psus[b][:], rz1[32 * b : 32 * b + 4, :]
                )
                # uT via 2 pack-transposes (4,128) -> (128,4)
                for h in range(2):
                    pstu = pstpool.tile([128, 4], F32R, tag="pst", name=f"pu{b}{h}")
                    nc.tensor.transpose(
                        pstu[:],
                        u_pack[:, 128 * h : 128 * h + 128],
                        eye_sb[0:4, 0:4],
                    )
                    nc.vector.tensor_copy(
                        uT[:, 8 * b + 4 * h : 8 * b + 4 * h + 4], pstu[:]
                    )
                # cavT[m] = sum_h W7b[h][:, m-chunk]^T @ uT[h]  -> (128, 4)
                for m in range(2):
                    cps = smpool.tile([128, 4], F32, tag="sm", name=f"cv{b}{m}")
                    for h in range(2):
                        nc.tensor.matmul(
                            cps[:],
                            w7b_sb[h][:, 128 * m : 128 * m + 128],
                            uT[:, 8 * b + 4 * h : 8 * b + 4 * h + 4],
                            start=(h == 0),
                            stop=(h == 1),
                        )
                    nc.vector.tensor_add(
                        v2t_sb[:, 16 * m + 4 * b : 16 * m + 4 * b + 4],
                        cps[:],
                        v1t_sb[:, 16 * m + 4 * b : 16 * m + 4 * b + 4],
                    )
                # (c) l2 logits for example b; evac = EXP act
                for g in range(NG):
                    wave = megapool.tile(
                        [4, 512], F32, tag="mega", name=f"wc{b}_{g}"
                    )
                    for dc in range(2):
                        nc.tensor.matmul(
                            wave[:],
                            v2t_sb[:, 16 * dc + 4 * b : 16 * dc + 4 * b + 4],
                            m_sb[b][dc][:, 512 * g : 512 * g + 512],
                            start=(dc == 0),
                            stop=(dc == 1),
                        )
                    nc.scalar.activation(
                        exp2_sb[32 * b : 32 * b + 4, 512 * g : 512 * g + 512],
                        wave[:],
                        EXP,
                    )

            def fin_out(which, exp_sb, bd, o_d):
                # fin rows 8b + g ; cols c -> n = 512g + c
                fin = megapool.tile([32, 512], F32, tag="mega", name=f"fin{which}")
                for g in range(NG):
                    nc.tensor.matmul(
                        fin[:],
                        bd[:, 32 * g : 32 * g + 32],
                        exp_sb[:, 512 * g : 512 * g + 512],
                        start=(g == 0),
                        stop=(g == NG - 1),
                    )
                o_sb = spool.tile([32, 512], F32, tag=f"o{which}", name=f"o{which}")
                nc.scalar.activation(o_sb[:], fin[:], LOG, scale=1.0 / PL)
                nc.sync.dma_start(
                    out=o_d.rearrange("b (g c) -> (b g) c", g=NG), in_=o_sb[:]
                )

            for b in range(BL):
                u_pass(b)
                if b > 0:
                    cav_and_c(b - 1)
                if b == 1:
                    # final p1 is independent of the (b)/(c) chain: fill the
                    # PE while u-passes wait on MT DMA
                    fin_out(0, exp1_sb, bd31, o1_d)
            cav_and_c(BL - 1)

            # ---- softmax2 normalizers + final p2 ----
            z2 = spool.tile([128, 1], F32, tag="z2")
            nc.vector.tensor_reduce(
                z2[:], exp2_sb[:], axis=mybir.AxisListType.X, op=mybir.AluOpType.add
            )
            rz2 = spool.tile([128, 1], F32, tag="rz2")
            nc.vector.reciprocal(rz2[:], z2[:])
            bd32 = spool.tile([128, 256], F32R, tag="bd32")
            for g in range(NG):
                nc.vector.tensor_scalar_mul(
                    bd32[:, 32 * g : 32 * g + 32],
                    p4g_sb[:, 32 * g : 32 * g + 32],
                    rz2[:],
                )
            fin_out(1, exp2_sb, bd32, o2_d)

    nc.compile()
    return nc


def _host_precompute(inp):
    H_q, M, W_4, W_6, W_7 = (
        inp["H_q"],
        inp["M"],
        inp["W_4"],
        inp["W_6"],
        inp["W_7"],
    )
    wih, whh, bih, bhh = (
        inp["gru_w_ih"],
        inp["gru_w_hh"],
        inp["gru_b_ih"],
        inp["gru_b_hh"],
    )
    lg = H_q @ W_4
    a = np.exp(lg - lg.max(1, keepdims=True))
    a /= a.sum(1, keepdims=True)
    s = np.einsum("bq,bqh->bh", a, H_q).astype(np.float32)
    x = M.mean(axis=2)
    gh = x @ whh.T + bhh
    ghr, ghz, ghn = np.split(gh, 3, axis=1)
    s_all = [s]
    for _ in range(T - 1):
        gi = s @ wih.T + bih
        gir, giz, gin = np.split(gi, 3, axis=1)
        r = 1.0 / (1.0 + np.exp(-(gir + ghr)))
        z = 1.0 / (1.0 + np.exp(-(giz + ghz)))
        n = np.tanh(gin + r * ghn)
        s = (1.0 - z) * n + z * x
        s_all.append(s)
    S = np.stack(s_all).astype(np.float32)  # (T, B, D2)
    SW6 = np.einsum("tbd,de->tbe", S, W_6).astype(np.float32)
    W7t, W7b = W_7[:D2], W_7[D2:]
    V1 = np.einsum("tbd,de->tbe", S, W7t).astype(np.float32)
    return SW6, V1, W7b


def kernel(**inputs):
    global _NC
    inp = {
        k: np.ascontiguousarray(np.asarray(v, dtype=np.float32))
        for k, v in inputs.items()
    }
    SW6, V1, W7b = _host_precompute(inp)
    M = inp["M"]

    eye = np.eye(128, dtype=np.float32)
    # p4g[:, 32g:32g+32][32b+t, 8b'+g'] = d(b=b') * d(g=g')  (t < 4 rows only)
    rows = np.arange(128)
    bb, tt = rows // 32, rows % 32
    p4g = np.zeros((128, 256), dtype=np.float32)
    for g in range(NG):
        valid = tt < 4
        p4g[rows[valid], 32 * g + 8 * bb[valid] + g] = 1.0
    w7b_h = np.ascontiguousarray(W7b.reshape(2, 128, 256))

    if _NC is None:
        _NC = _build_graph()
    in_maps = []
    for i in range(NCORES):
        sl = slice(i * BL, (i + 1) * BL)
        Mc = M[sl]  # (BL, 256, PL)
        # mt[b, kg, p, 256*kk + e] = M[b, e, 1024*kg + 128*kk + p]
        mt = np.ascontiguousarray(
            Mc.transpose(0, 2, 1)
            .reshape(BL, 4, 8, 128, 256)
            .transpose(0, 1, 3, 2, 4)
            .reshape(BL, 4, 128, 2048)
            .astype(ml_dtypes.bfloat16)
        )
        mc = np.ascontiguousarray(Mc.reshape(BL, 2, 128, PL))
        # sw6/v1t: [dc, d', 4b+t] = X[t, b, 128dc + d']
        sw6c = np.ascontiguousarray(
            SW6[:, sl].transpose(2, 1, 0).reshape(2, 128, 16)
        )
        v1c = np.ascontiguousarray(
            V1[:, sl].transpose(2, 1, 0).reshape(2, 128, 16)
        )
        in_maps.append(
            {
                "m": mc,
                "mt": mt,
                "sw6": sw6c,
                "v1t": v1c,
                "w7b": w7b_h,
                "eye": eye,
                "p4g": p4g,
            }
        )
    global _LAST_IN_MAPS
    _LAST_IN_MAPS = in_maps
    res = run_bass_kernel_spmd(_NC, in_maps, core_ids=list(range(NCORES)))
    out1 = np.empty((B, PL), np.float32)
    out2 = np.empty((B, PL), np.float32)
    for i in range(NCORES):
        out1[i * BL : (i + 1) * BL] = res.results[i]["o1"]
        out2[i * BL : (i + 1) * BL] = res.results[i]["o2"]
    return out1, out2


# revision 25
# speedup vs baseline: 1.3095x; 1.0605x over previous
"""Trainium2 Bass kernel for nn_AnswerModule (scatter_memory, 8 cores).

Strategy: pure data-parallel over batch (4 examples per core).  The
heavy einsums are collapsed via matmul associativity:
  p1 = softmax((s@W6) @ M),  attn@W7b = p1 @ (M^T @ W7b),
  p2 = softmax((s@W7t + attn@W7b) @ M).
The tiny GRU / alpha-attention recurrence runs on host.

Device dataflow (per core) is built around LONG moving-dim matmuls
(512 cols) with tiny stationaries to minimize PE instruction count
(per-instruction overhead on TRN2 is ~130-350 ns).  All partition
bases are 32-aligned (hardware requirement for every engine):
  (a) l1 logits: stationary = SW6 (128d x 4), moving = M (128d x 512),
      into (4,512) psum wave tiles; evacuation IS the exp: scalar
      activations write exp1[32b+t, n] with accum_out Z-partials.
  (b) attn u: host uploads M^T (n-major); stationary = exp1^T chunk
      slices (128n x 4) from 32 PE transposes, moving = M^T
      (128n x 256), accumulated over n into psum (4,256) per example.
  (c) l2 logits: same wave scheme as (a) with v2^T stationaries.
  Finals: 8 accumulating matmuls per output with per-block masked
  block-diag stationaries bd3_g[32b+t, 4b'+g'] = rz[b,t] d(b=b') d(g=g'),
  then one LOG activation and a partition-contiguous DMA.
"""

import sys

sys.path.insert(0, "/opt/trn_rl_repo")

import numpy as np
import ml_dtypes

import concourse.bass as bass
import concourse.bacc as bacc
import concourse.mybir as mybir
from concourse import tile
from concourse.bass_utils import run_bass_kernel_spmd

B, QL, PL, T, D2 = 32, 64, 4096, 4, 256
NCORES = 8
BL = B // NCORES  # 4 examples per core
NG = 8  # n-groups of 512
F32 = mybir.dt.float32
F32R = mybir.dt.float32r
BF16 = mybir.dt.bfloat16

_NC = None


def _build_graph():
    nc = bacc.Bacc("TRN2", target_bir_lowering=False, debug=False)

    # DRAM inputs
    m_d = nc.dram_tensor("m", [BL, 2, 128, PL], F32R, kind="ExternalInput").ap()
    mt_d = nc.dram_tensor("mt", [BL, 4, 128, 2048], BF16, kind="ExternalInput").ap()
    sw6_d = nc.dram_tensor("sw6", [2, 128, 16], F32R, kind="ExternalInput").ap()
    v1t_d = nc.dram_tensor("v1t", [2, 128, 16], F32R, kind="ExternalInput").ap()
    w7b_d = nc.dram_tensor("w7b", [2, 128, 256], F32R, kind="ExternalInput").ap()
    eye_d = nc.dram_tensor("eye", [128, 128], F32R, kind="ExternalInput").ap()
    p4g_d = nc.dram_tensor("p4g", [128, 256], F32R, kind="ExternalInput").ap()
    o1_d = nc.dram_tensor("o1", [BL, PL], F32, kind="ExternalOutput").ap()
    o2_d = nc.dram_tensor("o2", [BL, PL], F32, kind="ExternalOutput").ap()

    EXP = mybir.ActivationFunctionType.Exp
    LOG = getattr(mybir.ActivationFunctionType, "Log", None) or getattr(
        mybir.ActivationFunctionType, "Ln"
    )

    with tile.TileContext(nc) as tc:
        with (
            nc.allow_low_precision(reason="float32r is 4-byte, same width as f32"),
            tc.tile_pool(name="const", bufs=1) as cpool,
            tc.tile_pool(name="m", bufs=1) as mpool,
            tc.tile_pool(name="mt", bufs=6) as mtpool,
            tc.tile_pool(name="sb", bufs=1) as spool,
            tc.tile_pool(name="mega", bufs=3, space="PSUM") as megapool,
            tc.tile_pool(name="pst", bufs=2, space="PSUM") as pstpool,
            tc.tile_pool(name="psu", bufs=2, space="PSUM") as psupool,
            tc.tile_pool(name="sm", bufs=1, space="PSUM") as smpool,
        ):
            # ---- small constants ----
            sw6_sb = cpool.tile([128, 32], F32R, tag="sw6")
            v1t_sb = cpool.tile([128, 32], F32R, tag="v1t")
            for dc in range(2):
                nc.sync.dma_start(
                    out=sw6_sb[:, 16 * dc : 16 * dc + 16], in_=sw6_d[dc]
                )
                nc.sync.dma_start(
                    out=v1t_sb[:, 16 * dc : 16 * dc + 16], in_=v1t_d[dc]
                )
            w7b_sb = []
            for h in range(2):
                t_ = cpool.tile([128, 256], F32R, tag=f"w7b{h}", name=f"w7b{h}")
                nc.sync.dma_start(out=t_[:], in_=w7b_d[h])
                w7b_sb.append(t_)
            eye_sb = cpool.tile([128, 128], F32R, tag="eye")
            nc.sync.dma_start(out=eye_sb[:], in_=eye_d[:, :])
            p4g_sb = cpool.tile([128, 256], F32R, tag="p4g")
            nc.sync.dma_start(out=p4g_sb[:], in_=p4g_d[:, :])

            # exp stacks: rows 32b + t, cols n.  Junk rows zeroed once.
            exp1_sb = spool.tile([128, PL], F32R, tag="exp1")
            exp2_sb = spool.tile([128, PL], F32R, tag="exp2")
            nc.gpsimd.memset(exp1_sb[:].bitcast(F32), 1.0)
            nc.gpsimd.memset(exp2_sb[:].bitcast(F32), 1.0)

            # ---- M resident tiles (2MB each), then MT streaming tiles ----
            m_sb = [[None, None] for _ in range(BL)]
            for b in range(BL):
                for dc in range(2):
                    mtile = mpool.tile(
                        [128, PL], F32R, tag=f"m{b}_{dc}", name=f"m{b}_{dc}"
                    )
                    nc.sync.dma_start(out=mtile[:], in_=m_d[b, dc])
                    m_sb[b][dc] = mtile
            # MT: per example 4 tiles of (128, 8*256); issued after M so M
            # wins the DMA bandwidth early.
            mt_tiles = [[None] * 4 for _ in range(BL)]
            for b in range(BL):
                for kg in range(4):
                    t_ = mtpool.tile([128, 2048], BF16, tag="mt", name=f"mt{b}_{kg}")
                    nc.sync.dma_start(out=t_[:], in_=mt_d[b, kg])
                    mt_tiles[b][kg] = t_

            # ---- (a) l1 logits via psum wave tiles; evac = EXP act ----
            exp1T = spool.tile([128, PL], BF16, tag="exp1T")

            def a_wave(b, g):
                wave = megapool.tile([4, 512], F32, tag="mega", name=f"wa{b}_{g}")
                for dc in range(2):
                    nc.tensor.matmul(
                        wave[:],
                        sw6_sb[:, 16 * dc + 4 * b : 16 * dc + 4 * b + 4],
                        m_sb[b][dc][:, 512 * g : 512 * g + 512],
                        start=(dc == 0),
                        stop=(dc == 1),
                    )
                nc.scalar.activation(
                    exp1_sb[32 * b : 32 * b + 4, 512 * g : 512 * g + 512],
                    wave[:],
                    EXP,
                )

            def transpose_chunk(k):
                # T_k at exp1T[:, 128k:+128]; stationary for (b, chunk k)
                # = exp1T[:, 128k + 32b : +4]
                pst = pstpool.tile([128, 128], F32R, tag="pst", name=f"pst{k}")
                nc.tensor.transpose(
                    pst[:], exp1_sb[:, 128 * k : 128 * k + 128], eye_sb[:]
                )
                nc.vector.tensor_copy(exp1T[:, 128 * k : 128 * k + 128], pst[:])

            zp1c = spool.tile([128, NG], F32, tag="zp1c")
            for b in range(BL - 1):
                for g in range(NG):
                    a_wave(b, g)
            # last example: interleave the exp1^T transposes and the per-block
            # Z partial reduces per finished group
            for g in range(NG):
                a_wave(BL - 1, g)
                for j in range(4):
                    transpose_chunk(4 * g + j)
                nc.vector.tensor_reduce(
                    zp1c[:, g : g + 1],
                    exp1_sb[:, 512 * g : 512 * g + 512],
                    axis=mybir.AxisListType.X,
                    op=mybir.AluOpType.add,
                )

            # Z and 1/Z at rows 32b + t
            z1 = spool.tile([128, 1], F32, tag="z1")
            nc.vector.tensor_reduce(
                z1[:], zp1c[:], axis=mybir.AxisListType.X, op=mybir.AluOpType.add
            )
            rz1 = spool.tile([128, 1], F32, tag="rz1")
            nc.vector.reciprocal(rz1[:], z1[:])
            bd31 = spool.tile([128, 256], F32R, tag="bd31")
            nc.vector.tensor_scalar_mul(bd31[:], p4g_sb[:], rz1[:])

            # ---- per-example u (attn) / cav / l2, software-pipelined so the
            # PE never stalls on the small vector chain of the previous
            # example ----
            uT = spool.tile([128, 32], F32R, tag="uT")
            v2t_sb = spool.tile([128, 32], F32R, tag="v2t")
            zp2c = spool.tile([128, NG], F32, tag="zp2c")
            psus = [None] * BL

            def u_pass(b):
                # u_b[t, e] = sum_n exp1T_b[n, t] * MT_b[n, e]  -> (4, 256)
                psu = psupool.tile([4, 256], F32, tag="psu", name=f"psu{b}")
                for k in range(32):
                    kg, kk = k // 8, k % 8
                    nc.tensor.matmul(
                        psu[:],
                        exp1T[:, 128 * k + 32 * b : 128 * k + 32 * b + 4],
                        mt_tiles[b][kg][:, 256 * kk : 256 * kk + 256],
                        start=(k == 0),
                        stop=(k == 31),
                    )
                psus[b] = psu

            def cav_and_c(b):
                # extract + scale by 1/Z1 (double-buffered (4,256) tile)
                u_pack = spool.tile(
                    [4, 256], F32R, tag="u_pack", bufs=2, name=f"up{b}"
                )
                nc.vector.tensor_scalar_mul(
                    u_pack[:], psus[b][:], rz1[32 * b : 32 * b + 4, :]
                )
                # uT via 2 pack-transposes (4,128) -> (128,4)
                for h in range(2):
                    pstu = pstpool.tile([128, 4], F32R, tag="pst", name=f"pu{b}{h}")
                    nc.tensor.transpose(
                        pstu[:],
                        u_pack[:, 128 * h : 128 * h + 128],
                        eye_sb[0:4, 0:4],
                    )
                    nc.vector.tensor_copy(
                        uT[:, 8 * b + 4 * h : 8 * b + 4 * h + 4], pstu[:]
                    )
                # cavT[m] = sum_h W7b[h][:, m-chunk]^T @ uT[h]  -> (128, 4)
                for m in range(2):
                    cps = smpool.tile([128, 4], F32, tag="sm", name=f"cv{b}{m}")
                    for h in range(2):
                        nc.tensor.matmul(
                            cps[:],
                            w7b_sb[h][:, 128 * m : 128 * m + 128],
                            uT[:, 8 * b + 4 * h : 8 * b + 4 * h + 4],
                            start=(h == 0),
                            stop=(h == 1),
                        )
                    nc.vector.tensor_add(
                        v2t_sb[:, 16 * m + 4 * b : 16 * m + 4 * b + 4],
                        cps[:],
                        v1t_sb[:, 16 * m + 4 * b : 16 * m + 4 * b + 4],
                    )
                # (c) l2 logits for example b; evac = EXP act
                for g in range(NG):
                    wave = megapool.tile(
                        [4, 512], F32, tag="mega", name=f"wc{b}_{g}"
                    )
                    for dc in range(2):
                        nc.tensor.matmul(
                            wave[:],
                            v2t_sb[:, 16 * dc + 4 * b : 16 * dc + 4 * b + 4],
                            m_sb[b][dc][:, 512 * g : 512 * g + 512],
                            start=(dc == 0),
                            stop=(dc == 1),
                        )
                    nc.scalar.activation(
                        exp2_sb[32 * b : 32 * b + 4, 512 * g : 512 * g + 512],
                        wave[:],
                        EXP,
                    )
                    if b == BL - 1:
                        # all examples done for this block: Z2 partial
                        nc.vector.tensor_reduce(
                            zp2c[:, g : g + 1],
                            exp2_sb[:, 512 * g : 512 * g + 512],
                            axis=mybir.AxisListType.X,
                            op=mybir.AluOpType.add,
                        )

            def fin_out(which, exp_sb, bd, o_d):
                # fin rows 8b + g ; cols c -> n = 512g + c
                fin = megapool.tile([32, 512], F32, tag="mega", name=f"fin{which}")
                for g in range(NG):
                    nc.tensor.matmul(
                        fin[:],
                        bd[:, 32 * g : 32 * g + 32],
                        exp_sb[:, 512 * g : 512 * g + 512],
                        start=(g == 0),
                        stop=(g == NG - 1),
                    )
                o_sb = spool.tile([32, 512], F32, tag=f"o{which}", name=f"o{which}")
                nc.scalar.activation(o_sb[:], fin[:], LOG, scale=1.0 / PL)
                nc.sync.dma_start(
                    out=o_d.rearrange("b (g c) -> (b g) c", g=NG), in_=o_sb[:]
                )

            for b in range(BL):
                u_pass(b)
                if b > 0:
                    cav_and_c(b - 1)
                if b == 1:
                    # final p1 is independent of the (b)/(c) chain: fill the
                    # PE while u-passes wait on MT DMA
                    fin_out(0, exp1_sb, bd31, o1_d)
            cav_and_c(BL - 1)

            # ---- softmax2 normalizers + final p2 ----
            z2 = spool.tile([128, 1], F32, tag="z2")
            nc.vector.tensor_reduce(
                z2[:], zp2c[:], axis=mybir.AxisListType.X, op=mybir.AluOpType.add
            )
            rz2 = spool.tile([128, 1], F32, tag="rz2")
            nc.vector.reciprocal(rz2[:], z2[:])
            bd32 = spool.tile([128, 256], F32R, tag="bd32")
            nc.vector.tensor_scalar_mul(bd32[:], p4g_sb[:], rz2[:])
            fin_out(1, exp2_sb, bd32, o2_d)

    nc.compile()
    return nc


def _host_precompute(inp):
    H_q, M, W_4, W_6, W_7 = (
        inp["H_q"],
        inp["M"],
        inp["W_4"],
        inp["W_6"],
        inp["W_7"],
    )
    wih, whh, bih, bhh = (
        inp["gru_w_ih"],
        inp["gru_w_hh"],
        inp["gru_b_ih"],
        inp["gru_b_hh"],
    )
    lg = H_q @ W_4
    a = np.exp(lg - lg.max(1, keepdims=True))
    a /= a.sum(1, keepdims=True)
    s = np.einsum("bq,bqh->bh", a, H_q).astype(np.float32)
    x = M.mean(axis=2)
    gh = x @ whh.T + bhh
    ghr, ghz, ghn = np.split(gh, 3, axis=1)
    s_all = [s]
    for _ in range(T - 1):
        gi = s @ wih.T + bih
        gir, giz, gin = np.split(gi, 3, axis=1)
        r = 1.0 / (1.0 + np.exp(-(gir + ghr)))
        z = 1.0 / (1.0 + np.exp(-(giz + ghz)))
        n = np.tanh(gin + r * ghn)
        s = (1.0 - z) * n + z * x
        s_all.append(s)
    S = np.stack(s_all).astype(np.float32)  # (T, B, D2)
    SW6 = np.einsum("tbd,de->tbe", S, W_6).astype(np.float32)
    W7t, W7b = W_7[:D2], W_7[D2:]
    V1 = np.einsum("tbd,de->tbe", S, W7t).astype(np.float32)
    return SW6, V1, W7b


def kernel(**inputs):
    global _NC
    inp = {
        k: np.ascontiguousarray(np.asarray(v, dtype=np.float32))
        for k, v in inputs.items()
    }
    SW6, V1, W7b = _host_precompute(inp)
    M = inp["M"]

    eye = np.eye(128, dtype=np.float32)
    # p4g[:, 32g:32g+32][32b+t, 8b'+g'] = d(b=b') * d(g=g')  (t < 4 rows only)
    rows = np.arange(128)
    bb, tt = rows // 32, rows % 32
    p4g = np.zeros((128, 256), dtype=np.float32)
    for g in range(NG):
        valid = tt < 4
        p4g[rows[valid], 32 * g + 8 * bb[valid] + g] = 1.0
    w7b_h = np.ascontiguousarray(W7b.reshape(2, 128, 256))

    if _NC is None:
        _NC = _build_graph()
    in_maps = []
    for i in range(NCORES):
        sl = slice(i * BL, (i + 1) * BL)
        Mc = M[sl]  # (BL, 256, PL)
        # mt[b, kg, p, 256*kk + e] = M[b, e, 1024*kg + 128*kk + p]
        mt = np.ascontiguousarray(
            Mc.transpose(0, 2, 1)
            .reshape(BL, 4, 8, 128, 256)
            .transpose(0, 1, 3, 2, 4)
            .reshape(BL, 4, 128, 2048)
            .astype(ml_dtypes.bfloat16)
        )
        mc = np.ascontiguousarray(Mc.reshape(BL, 2, 128, PL))
        # sw6/v1t: [dc, d', 4b+t] = X[t, b, 128dc + d']
        sw6c = np.ascontiguousarray(
            SW6[:, sl].transpose(2, 1, 0).reshape(2, 128, 16)
        )
        v1c = np.ascontiguousarray(
            V1[:, sl].transpose(2, 1, 0).reshape(2, 128, 16)
        )
        in_maps.append(
            {
                "m": mc,
                "mt": mt,
                "sw6": sw6c,
                "v1t": v1c,
                "w7b": w7b_h,
                "eye": eye,
                "p4g": p4g,
            }
        )
    global _LAST_IN_MAPS
    _LAST_IN_MAPS = in_maps
    res = run_bass_kernel_spmd(_NC, in_maps, core_ids=list(range(NCORES)))
    out1 = np.empty((B, PL), np.float32)
    out2 = np.empty((B, PL), np.float32)
    for i in range(NCORES):
        out1[i * BL : (i + 1) * BL] = res.results[i]["o1"]
        out2[i * BL : (i + 1) * BL] = res.results[i]["o2"]
    return out1, out2


# revision 26
# speedup vs baseline: 1.5265x; 1.1657x over previous
"""Trainium2 Bass kernel for nn_AnswerModule (scatter_memory, 8 cores).

Strategy: pure data-parallel over batch (4 examples per core).  The
heavy einsums are collapsed via matmul associativity:
  p1 = softmax((s@W6) @ M),  attn@W7b = p1 @ (M^T @ W7b),
  p2 = softmax((s@W7t + attn@W7b) @ M).
The tiny GRU / alpha-attention recurrence runs on host.

Device dataflow (per core) is built around LONG moving-dim matmuls
(512 cols) with tiny stationaries to minimize PE instruction count
(per-instruction overhead on TRN2 is ~130-350 ns).  All partition
bases are 32-aligned (hardware requirement for every engine):
  (a) l1 logits: stationary = SW6 (128d x 4), moving = M (128d x 512),
      into (4,512) psum wave tiles; evacuation IS the exp: scalar
      activations write exp1[32b+t, n] with accum_out Z-partials.
  (b) attn u: host uploads M^T (n-major); stationary = exp1^T chunk
      slices (128n x 4) from 32 PE transposes, moving = M^T
      (128n x 256), accumulated over n into psum (4,256) per example.
  (c) l2 logits: same wave scheme as (a) with v2^T stationaries.
  Finals: 8 accumulating matmuls per output with per-block masked
  block-diag stationaries bd3_g[32b+t, 4b'+g'] = rz[b,t] d(b=b') d(g=g'),
  then one LOG activation and a partition-contiguous DMA.
"""

import sys

sys.path.insert(0, "/opt/trn_rl_repo")

import numpy as np
import ml_dtypes

import concourse.bass as bass
import concourse.bacc as bacc
import concourse.mybir as mybir
from concourse import tile
from concourse.bass_utils import run_bass_kernel_spmd

B, QL, PL, T, D2 = 32, 64, 4096, 4, 256
NCORES = 8
BL = B // NCORES  # 4 examples per core
NG = 8  # n-groups of 512
F32 = mybir.dt.float32
F32R = mybir.dt.float32r
BF16 = mybir.dt.bfloat16
FP16 = mybir.dt.float16

_NC = None


def _build_graph():
    nc = bacc.Bacc("TRN2", target_bir_lowering=False, debug=False)

    # DRAM inputs
    m_d = nc.dram_tensor("m", [BL, 2, 128, PL], FP16, kind="ExternalInput").ap()
    mt_d = nc.dram_tensor("mt", [BL, 4, 128, 2048], BF16, kind="ExternalInput").ap()
    sw6_d = nc.dram_tensor("sw6", [2, 128, 16], FP16, kind="ExternalInput").ap()
    v1t_d = nc.dram_tensor("v1t", [2, 128, 16], FP16, kind="ExternalInput").ap()
    w7b_d = nc.dram_tensor("w7b", [2, 128, 256], F32R, kind="ExternalInput").ap()
    eye_d = nc.dram_tensor("eye", [128, 128], F32R, kind="ExternalInput").ap()
    p4g_d = nc.dram_tensor("p4g", [128, 256], F32R, kind="ExternalInput").ap()
    o1_d = nc.dram_tensor("o1", [BL, PL], F32, kind="ExternalOutput").ap()
    o2_d = nc.dram_tensor("o2", [BL, PL], F32, kind="ExternalOutput").ap()

    EXP = mybir.ActivationFunctionType.Exp
    LOG = getattr(mybir.ActivationFunctionType, "Log", None) or getattr(
        mybir.ActivationFunctionType, "Ln"
    )

    with tile.TileContext(nc) as tc:
        with (
            nc.allow_low_precision(reason="float32r is 4-byte, same width as f32"),
            tc.tile_pool(name="const", bufs=1) as cpool,
            tc.tile_pool(name="m", bufs=1) as mpool,
            tc.tile_pool(name="mt", bufs=6) as mtpool,
            tc.tile_pool(name="sb", bufs=1) as spool,
            tc.tile_pool(name="mega", bufs=3, space="PSUM") as megapool,
            tc.tile_pool(name="pst", bufs=2, space="PSUM") as pstpool,
            tc.tile_pool(name="psu", bufs=2, space="PSUM") as psupool,
            tc.tile_pool(name="sm", bufs=1, space="PSUM") as smpool,
        ):
            # ---- small constants ----
            sw6_sb = cpool.tile([128, 32], FP16, tag="sw6")
            v1t_sb = cpool.tile([128, 32], FP16, tag="v1t")
            for dc in range(2):
                nc.sync.dma_start(
                    out=sw6_sb[:, 16 * dc : 16 * dc + 16], in_=sw6_d[dc]
                )
                nc.sync.dma_start(
                    out=v1t_sb[:, 16 * dc : 16 * dc + 16], in_=v1t_d[dc]
                )
            w7b_sb = []
            for h in range(2):
                t_ = cpool.tile([128, 256], F32R, tag=f"w7b{h}", name=f"w7b{h}")
                nc.sync.dma_start(out=t_[:], in_=w7b_d[h])
                w7b_sb.append(t_)
            eye_sb = cpool.tile([128, 128], F32R, tag="eye")
            nc.sync.dma_start(out=eye_sb[:], in_=eye_d[:, :])
            p4g_sb = cpool.tile([128, 256], F32R, tag="p4g")
            nc.sync.dma_start(out=p4g_sb[:], in_=p4g_d[:, :])

            # preload the LOG activation table during the DMA shadow
            scr = cpool.tile([32, 4], F32, tag="scr")
            nc.scalar.activation(scr[:], eye_sb[0:32, 0:4].bitcast(F32), LOG)

            # exp stacks: rows 32b + t, cols n.  Junk rows zeroed once.
            exp1_sb = spool.tile([128, PL], F32R, tag="exp1")
            exp2_sb = spool.tile([128, PL], F32R, tag="exp2")
            nc.gpsimd.memset(exp1_sb[:].bitcast(F32), 1.0)
            nc.gpsimd.memset(exp2_sb[:].bitcast(F32), 1.0)

            # ---- M resident tiles (2MB each), then MT streaming tiles ----
            m_sb = [[None, None] for _ in range(BL)]
            for b in range(BL):
                for dc in range(2):
                    mtile = mpool.tile(
                        [128, PL], FP16, tag=f"m{b}_{dc}", name=f"m{b}_{dc}"
                    )
                    nc.sync.dma_start(out=mtile[:], in_=m_d[b, dc])
                    m_sb[b][dc] = mtile
            # MT: per example 4 tiles of (128, 8*256); issued after M so M
            # wins the DMA bandwidth early.
            mt_tiles = [[None] * 4 for _ in range(BL)]
            for b in range(BL):
                for kg in range(4):
                    t_ = mtpool.tile([128, 2048], BF16, tag="mt", name=f"mt{b}_{kg}")
                    nc.sync.dma_start(out=t_[:], in_=mt_d[b, kg])
                    mt_tiles[b][kg] = t_

            # ---- (a) l1 logits via psum wave tiles; evac = EXP act ----
            exp1T = spool.tile([128, PL], BF16, tag="exp1T")

            def a_wave(b, g):
                # two g-groups share each stationary load
                waves = [
                    megapool.tile([4, 512], F32, tag="mega", name=f"wa{b}_{g}_{i}")
                    for i in range(2)
                ]
                for dc in range(2):
                    for i in range(2):
                        nc.tensor.matmul(
                            waves[i][:],
                            sw6_sb[:, 16 * dc + 4 * b : 16 * dc + 4 * b + 4],
                            m_sb[b][dc][:, 512 * (g + i) : 512 * (g + i) + 512],
                            start=(dc == 0),
                            stop=(dc == 1),
                        )
                for i in range(2):
                    nc.scalar.activation(
                        exp1_sb[
                            32 * b : 32 * b + 4,
                            512 * (g + i) : 512 * (g + i) + 512,
                        ],
                        waves[i][:],
                        EXP,
                    )

            def transpose_chunk(k):
                # T_k at exp1T[:, 128k:+128]; stationary for (b, chunk k)
                # = exp1T[:, 128k + 32b : +4]
                pst = pstpool.tile([128, 128], F32R, tag="pst", name=f"pst{k}")
                nc.tensor.transpose(
                    pst[:], exp1_sb[:, 128 * k : 128 * k + 128], eye_sb[:]
                )
                nc.vector.tensor_copy(exp1T[:, 128 * k : 128 * k + 128], pst[:])

            zp1c = spool.tile([128, NG], F32, tag="zp1c")
            for b in range(BL - 1):
                for g in range(0, NG, 2):
                    a_wave(b, g)
            # last example: interleave the exp1^T transposes and the per-block
            # Z partial reduces per finished group-pair
            for g in range(0, NG, 2):
                a_wave(BL - 1, g)
                for j in range(8):
                    transpose_chunk(4 * g + j)
                nc.vector.tensor_reduce(
                    zp1c[:, g : g + 2],
                    exp1_sb[:, 512 * g : 512 * g + 1024].rearrange(
                        "p (q c) -> p q c", q=2
                    ),
                    axis=mybir.AxisListType.X,
                    op=mybir.AluOpType.add,
                )

            # Z and 1/Z at rows 32b + t
            z1 = spool.tile([128, 1], F32, tag="z1")
            nc.vector.tensor_reduce(
                z1[:], zp1c[:], axis=mybir.AxisListType.X, op=mybir.AluOpType.add
            )
            rz1 = spool.tile([128, 1], F32, tag="rz1")
            nc.vector.reciprocal(rz1[:], z1[:])
            bd31 = spool.tile([128, 256], F32R, tag="bd31")
            nc.vector.tensor_scalar_mul(bd31[:], p4g_sb[:], rz1[:])

            # ---- per-example u (attn) / cav / l2, software-pipelined so the
            # PE never stalls on the small vector chain of the previous
            # example ----
            uT = spool.tile([128, 32], F32R, tag="uT")
            v2t_sb = spool.tile([128, 32], FP16, tag="v2t")
            zp2c = spool.tile([128, NG], F32, tag="zp2c")
            psus = [None] * BL

            def u_pass(b):
                # u_b[t, e] = sum_n exp1T_b[n, t] * MT_b[n, e]  -> (4, 256)
                psu = psupool.tile([4, 256], F32, tag="psu", name=f"psu{b}")
                for k in range(32):
                    kg, kk = k // 8, k % 8
                    nc.tensor.matmul(
                        psu[:],
                        exp1T[:, 128 * k + 32 * b : 128 * k + 32 * b + 4],
                        mt_tiles[b][kg][:, 256 * kk : 256 * kk + 256],
                        start=(k == 0),
                        stop=(k == 31),
                    )
                psus[b] = psu

            def cav_and_c(b):
                # extract + scale by 1/Z1 (double-buffered (4,256) tile)
                u_pack = spool.tile(
                    [4, 256], F32R, tag="u_pack", bufs=2, name=f"up{b}"
                )
                nc.vector.tensor_scalar_mul(
                    u_pack[:], psus[b][:], rz1[32 * b : 32 * b + 4, :]
                )
                # uT via 2 pack-transposes (4,128) -> (128,4)
                for h in range(2):
                    pstu = pstpool.tile([128, 4], F32R, tag="pst", name=f"pu{b}{h}")
                    nc.tensor.transpose(
                        pstu[:],
                        u_pack[:, 128 * h : 128 * h + 128],
                        eye_sb[0:4, 0:4],
                    )
                    nc.vector.tensor_copy(
                        uT[:, 8 * b + 4 * h : 8 * b + 4 * h + 4], pstu[:]
                    )
                # cavT[m] = sum_h W7b[h][:, m-chunk]^T @ uT[h]  -> (128, 4)
                for m in range(2):
                    cps = smpool.tile([128, 4], F32, tag="sm", name=f"cv{b}{m}")
                    for h in range(2):
                        nc.tensor.matmul(
                            cps[:],
                            w7b_sb[h][:, 128 * m : 128 * m + 128],
                            uT[:, 8 * b + 4 * h : 8 * b + 4 * h + 4],
                            start=(h == 0),
                            stop=(h == 1),
                        )
                    nc.vector.tensor_add(
                        v2t_sb[:, 16 * m + 4 * b : 16 * m + 4 * b + 4],
                        cps[:],
                        v1t_sb[:, 16 * m + 4 * b : 16 * m + 4 * b + 4],
                    )
                # (c) l2 logits for example b; evac = EXP act
                for g in range(0, NG, 2):
                    waves = [
                        megapool.tile(
                            [4, 512], F32, tag="mega", name=f"wc{b}_{g}_{i}"
                        )
                        for i in range(2)
                    ]
                    for dc in range(2):
                        for i in range(2):
                            nc.tensor.matmul(
                                waves[i][:],
                                v2t_sb[:, 16 * dc + 4 * b : 16 * dc + 4 * b + 4],
                                m_sb[b][dc][:, 512 * (g + i) : 512 * (g + i) + 512],
                                start=(dc == 0),
                                stop=(dc == 1),
                            )
                    for i in range(2):
                        nc.scalar.activation(
                            exp2_sb[
                                32 * b : 32 * b + 4,
                                512 * (g + i) : 512 * (g + i) + 512,
                            ],
                            waves[i][:],
                            EXP,
                        )
                    if b == BL - 1:
                        # all examples done for these blocks: Z2 partials
                        nc.vector.tensor_reduce(
                            zp2c[:, g : g + 2],
                            exp2_sb[:, 512 * g : 512 * g + 1024].rearrange(
                                "p (q c) -> p q c", q=2
                            ),
                            axis=mybir.AxisListType.X,
                            op=mybir.AluOpType.add,
                        )

            def fin_out(which, exp_sb, bd, o_d):
                # fin rows 8b + g ; cols c -> n = 512g + c
                fin = megapool.tile([32, 512], F32, tag="mega", name=f"fin{which}")
                for g in range(NG):
                    nc.tensor.matmul(
                        fin[:],
                        bd[:, 32 * g : 32 * g + 32],
                        exp_sb[:, 512 * g : 512 * g + 512],
                        start=(g == 0),
                        stop=(g == NG - 1),
                    )
                o_sb = spool.tile([32, 512], F32, tag=f"o{which}", name=f"o{which}")
                nc.scalar.activation(o_sb[:], fin[:], LOG, scale=1.0 / PL)
                nc.sync.dma_start(
                    out=o_d.rearrange("b (g c) -> (b g) c", g=NG), in_=o_sb[:]
                )

            for b in range(BL):
                u_pass(b)
                if b > 0:
                    cav_and_c(b - 1)
                if b == 1:
                    # final p1 is independent of the (b)/(c) chain: fill the
                    # PE while u-passes wait on MT DMA
                    fin_out(0, exp1_sb, bd31, o1_d)
            cav_and_c(BL - 1)

            # ---- softmax2 normalizers + final p2 ----
            z2 = spool.tile([128, 1], F32, tag="z2")
            nc.vector.tensor_reduce(
                z2[:], zp2c[:], axis=mybir.AxisListType.X, op=mybir.AluOpType.add
            )
            rz2 = spool.tile([128, 1], F32, tag="rz2")
            nc.vector.reciprocal(rz2[:], z2[:])
            bd32 = spool.tile([128, 256], F32R, tag="bd32")
            nc.vector.tensor_scalar_mul(bd32[:], p4g_sb[:], rz2[:])
            fin_out(1, exp2_sb, bd32, o2_d)

    nc.compile()
    return nc


def _host_precompute(inp):
    H_q, M, W_4, W_6, W_7 = (
        inp["H_q"],
        inp["M"],
        inp["W_4"],
        inp["W_6"],
        inp["W_7"],
    )
    wih, whh, bih, bhh = (
        inp["gru_w_ih"],
        inp["gru_w_hh"],
        inp["gru_b_ih"],
        inp["gru_b_hh"],
    )
    lg = H_q @ W_4
    a = np.exp(lg - lg.max(1, keepdims=True))
    a /= a.sum(1, keepdims=True)
    s = np.einsum("bq,bqh->bh", a, H_q).astype(np.float32)
    x = M.mean(axis=2)
    gh = x @ whh.T + bhh
    ghr, ghz, ghn = np.split(gh, 3, axis=1)
    s_all = [s]
    for _ in range(T - 1):
        gi = s @ wih.T + bih
        gir, giz, gin = np.split(gi, 3, axis=1)
        r = 1.0 / (1.0 + np.exp(-(gir + ghr)))
        z = 1.0 / (1.0 + np.exp(-(giz + ghz)))
        n = np.tanh(gin + r * ghn)
        s = (1.0 - z) * n + z * x
        s_all.append(s)
    S = np.stack(s_all).astype(np.float32)  # (T, B, D2)
    SW6 = np.einsum("tbd,de->tbe", S, W_6).astype(np.float32)
    W7t, W7b = W_7[:D2], W_7[D2:]
    V1 = np.einsum("tbd,de->tbe", S, W7t).astype(np.float32)
    return SW6, V1, W7b


def kernel(**inputs):
    global _NC
    inp = {
        k: np.ascontiguousarray(np.asarray(v, dtype=np.float32))
        for k, v in inputs.items()
    }
    SW6, V1, W7b = _host_precompute(inp)
    M = inp["M"]

    eye = np.eye(128, dtype=np.float32)
    # p4g[:, 32g:32g+32][32b+t, 8b'+g'] = d(b=b') * d(g=g')  (t < 4 rows only)
    rows = np.arange(128)
    bb, tt = rows // 32, rows % 32
    p4g = np.zeros((128, 256), dtype=np.float32)
    for g in range(NG):
        valid = tt < 4
        p4g[rows[valid], 32 * g + 8 * bb[valid] + g] = 1.0
    w7b_h = np.ascontiguousarray(W7b.reshape(2, 128, 256))

    if _NC is None:
        _NC = _build_graph()
    in_maps = []
    for i in range(NCORES):
        sl = slice(i * BL, (i + 1) * BL)
        Mc = M[sl]  # (BL, 256, PL)
        # mt[b, kg, p, 256*kk + e] = M[b, e, 1024*kg + 128*kk + p]
        mt = np.ascontiguousarray(
            Mc.transpose(0, 2, 1)
            .reshape(BL, 4, 8, 128, 256)
            .transpose(0, 1, 3, 2, 4)
            .reshape(BL, 4, 128, 2048)
            .astype(ml_dtypes.bfloat16)
        )
        mc = np.ascontiguousarray(Mc.reshape(BL, 2, 128, PL).astype(np.float16))
        # sw6/v1t: [dc, d', 4b+t] = X[t, b, 128dc + d']
        sw6c = np.ascontiguousarray(
            SW6[:, sl].transpose(2, 1, 0).reshape(2, 128, 16).astype(np.float16)
        )
        v1c = np.ascontiguousarray(
            V1[:, sl].transpose(2, 1, 0).reshape(2, 128, 16).astype(np.float16)
        )
        in_maps.append(
            {
                "m": mc,
                "mt": mt,
                "sw6": sw6c,
                "v1t": v1c,
                "w7b": w7b_h,
                "eye": eye,
                "p4g": p4g,
            }
        )
    global _LAST_IN_MAPS
    _LAST_IN_MAPS = in_maps
    res = run_bass_kernel_spmd(_NC, in_maps, core_ids=list(range(NCORES)))
    out1 = np.empty((B, PL), np.float32)
    out2 = np.empty((B, PL), np.float32)
    for i in range(NCORES):
        out1[i * BL : (i + 1) * BL] = res.results[i]["o1"]
        out2[i * BL : (i + 1) * BL] = res.results[i]["o2"]
    return out1, out2
